# revision 9
# baseline (speedup 1.0000x reference)
"""Bipolar morphological conv2d for Trainium2 (8 NeuronCores) via p-norm
soft-max on the PE.

Math: y = m(lp1,K1) - m(lp1,K2) - m(lp2,K1) + m(lp2,K2) + bias with
m(logp,k)[c] = exp(max_p(logp_p + k_pc)) = max(U_c, max_p(w_p * K_pc)),
K = exp(k), U_c = .1 max_p K_pc, w = relu(+-x) (entries below U are
subsumed by the U clamp, applied at the end in log domain).

The max over p=288 is approximated by a power-64 p-norm computed as a
MATMUL over host-prepared w^64 patches: S = sum_p (a v_p)^64.  Accuracy
is recovered by a two-term solve using per-tap partial sums F_t:
p1 = sum F_t, p2 = sum F_t^2 (~S128 up to same-tap ties), then the top
term a solves a+b=p1, a^2+b^2=p2: a = (p1 + sqrt(2 p2 - p1^2))/2, and
m = a^(1/64)/alpha.  A second scale band (w clipped at 0.3, plain
p-norm) covers small maxima that underflow the main band; bands merge
in log domain with the exact ln(U) floor.  Simulated end-to-end error
vs the reference: rel L2 ~1.0e-2 (budget 2e-2).

Device (per core = one batch image; partitions = [64c K1 | 64c K2]):
  PE:  per (sign, tap, half): K=32 matmuls vs bf16 w^64 window rows ->
       F_t in PSUM; accumulated passes for S_hi / S_lo.
  ACT: squares F_t into bf16 (scale 1e-19), Ln / Sqrt / Exp of the solve.
  DVE: sums of squares (bf16 2x), the solve arithmetic, band merge.
  PE:  final combine (+-I transpose matmuls) + bias, DMA out.
Host precomputes all powers/scales (input-only transforms), so the
device never exponentiates x.
"""

import os
from contextlib import ExitStack

import numpy as np
import ml_dtypes

import concourse.bass as bass
import concourse.mybir as mybir
from concourse import bacc
import concourse.tile as tile
from concourse.bass_utils import run_bass_kernel_spmd

N_CORES = 8
H = W = W_ = C = 32
COUT = 64
HO = WO = 30
NPIX = H * W            # 1024
FD = HO * WO            # 900
XROW = 1056             # padded power-row length
Q = 64
LAM = 1e-19             # F rescale inside ACT Square
XT_TOP = 1e33           # x-side bf16 top target
PT_TOP = 1e36           # per-product fp32 top target
WMAX = 4.8              # |x| bound
WCLIP = 0.3             # low-band clip
HALF = 450              # positions per PSUM half (15 rows)

F32 = mybir.dt.float32
F16 = mybir.dt.float16
BF16 = mybir.dt.bfloat16
_cache: dict = {}
last_results = None


def _ensure_axon_ntff_hook():
    import sys
    import types

    try:
        import antenv.axon_hooks  # noqa: F401
        return
    except ImportError:
        pass
    try:
        mod = types.ModuleType("antenv.axon_hooks")
        holder = [None]
        mod.set_axon_ntff_profile_hook = lambda h: holder.__setitem__(0, h)
        mod.get_axon_ntff_profile_hook = lambda: holder[0]
        sys.modules["antenv.axon_hooks"] = mod
        from trn_agent_boot.trn_boot import _ntff_profile_via_ctypes

        so = "/opt/axon/libaxon_pjrt.so"
        if os.path.exists(so):
            holder[0] = _ntff_profile_via_ctypes(so)
    except Exception:
        pass


def _build_module():
    nc = bacc.Bacc()
    Alu = mybir.AluOpType
    Act = mybir.ActivationFunctionType

    # power rows: per sign/band, even+odd parity copies [C, XROW] bf16
    drams = {}
    for nm in ("PHAe", "PHAo", "PHBe", "PHBo"):
        drams[nm] = nc.dram_tensor(nm, [C, XROW], BF16, kind="ExternalInput")
    for nm in ("WHA", "WHB", "WLA", "WLB"):
        drams[nm] = nc.dram_tensor(nm, [96, 3 * 960], BF16, kind="ExternalInput")
    KQ = nc.dram_tensor("KQ", [C, 9 * 128], BF16, kind="ExternalInput")
    KQ96 = nc.dram_tensor("KQ96", [96, 3 * 128], BF16, kind="ExternalInput")
    SC = nc.dram_tensor("SC", [128, 3], F32, kind="ExternalInput")  # b_hi, b_lo, lnU
    M1 = nc.dram_tensor("M1", [128, COUT], F16, kind="ExternalInput")
    M2 = nc.dram_tensor("M2", [128, COUT], F16, kind="ExternalInput")
    BC = nc.dram_tensor("BC", [128, COUT], F32, kind="ExternalInput")
    Y = nc.dram_tensor("Y", [FD, COUT], F32, kind="ExternalOutput")

    with tile.TileContext(nc) as tc, ExitStack() as ctx:
        const = ctx.enter_context(tc.tile_pool(name="const", bufs=1))
        psb = ctx.enter_context(tc.tile_pool(name="psb", bufs=6, space="PSUM"))
        psc = ctx.enter_context(tc.tile_pool(name="psc", bufs=2, space="PSUM"))
        gst = ctx.enter_context(tc.tile_pool(name="gst", bufs=2))
        wrk = ctx.enter_context(tc.tile_pool(name="wrk", bufs=2))
        mtp = ctx.enter_context(tc.tile_pool(name="mtp", bufs=2))
        tsb = ctx.enter_context(tc.tile_pool(name="tsb", bufs=2))

        xs = {}
        for i, nm in enumerate(("PHAe", "PHAo", "PHBe", "PHBo")):
            t = const.tile([C, XROW], BF16, name=nm, tag=nm)
            eng = nc.sync if i % 2 == 0 else nc.gpsimd
            eng.dma_start(out=t[:, :], in_=drams[nm][:, :])
            xs[nm] = t
        for i, nm in enumerate(("WHA", "WHB", "WLA", "WLB")):
            t = const.tile([96, 3 * 960], BF16, name=nm, tag=nm)
            eng = nc.sync if i % 2 == 0 else nc.gpsimd
            eng.dma_start(out=t[:, :], in_=drams[nm][:, :])
            xs[nm] = t
        KQ_sb = const.tile([C, 9 * 128], BF16)
        nc.gpsimd.dma_start(out=KQ_sb[:, :], in_=KQ[:, :])
        KQ96_sb = const.tile([96, 3 * 128], BF16)
        nc.gpsimd.dma_start(out=KQ96_sb[:, :], in_=KQ96[:, :])
        SC_sb = const.tile([128, 3], F32)
        nc.gpsimd.dma_start(out=SC_sb[:, :], in_=SC[:, :])
        M1_sb = const.tile([128, COUT], F16)
        nc.gpsimd.dma_start(out=M1_sb[:, :], in_=M1[:, :])
        M2_sb = const.tile([128, COUT], F16)
        nc.gpsimd.dma_start(out=M2_sb[:, :], in_=M2[:, :])
        BC_sb = const.tile([128, COUT], F32)
        nc.gpsimd.dma_start(out=BC_sb[:, :], in_=BC[:, :])

        def rhs_win(sign, band, t, h):
            """window AP [32, 15, 30] for tap t, half h."""
            i, j = divmod(t, 3)
            off = i * W + j
            key = ("PH" if band == "hi" else "PL") + sign
            if off % 2 == 0:
                src, base = xs[key + "e"], off
            else:
                src, base = xs[key + "o"], off - 1
            base += h * 15 * W
            return src[:, base:base + 480].rearrange(
                "q (a b) -> q a b", b=W)[:, :15, :WO]

        m_out = {}
        SHs, Gs, LLOs, los = {}, {}, {}, {}
        # phase 1: accumulated S passes (hi), staged via DVE with the lam scale
        for sign in ("A", "B"):
            p1 = wrk.tile([128, FD], F32, tag="p1" + sign)
            Wx = xs["WH" + sign]
            for h in range(2):
                sp = psb.tile([128, 512], F32, tag="ps")
                for g in range(3):
                    nc.tensor.matmul(
                        sp[:, 0:HALF],
                        lhsT=KQ96_sb[:, g * 128:(g + 1) * 128],
                        rhs=Wx[:, g * 960 + h * 480:g * 960 + h * 480 + 480
                              ].rearrange("q (a b) -> q a b", b=W_)[:, :15, :WO],
                        start=(g == 0), stop=(g == 2))
                nc.vector.tensor_scalar(
                    out=p1[:, h * HALF:(h + 1) * HALF], in0=sp[:, 0:HALF],
                    scalar1=float(LAM), scalar2=None, op0=Alu.mult)
            SHs[sign] = p1
        # phase 2: per-tap F passes, ACT Square only (one table load)
        for sign in ("A", "B"):
            G = gst.tile([128, 9 * FD], BF16, tag="G" + sign)
            for t in range(9):
                for h in range(2):
                    fp = psb.tile([128, 512], F32, tag="ps")
                    nc.tensor.matmul(
                        fp[:, 0:HALF],
                        lhsT=KQ_sb[:, t * 128:(t + 1) * 128],
                        rhs=rhs_win(sign, "hi", t, h),
                        start=True, stop=True)
                    nc.scalar.activation(
                        out=G[:, t * FD + h * HALF:t * FD + (h + 1) * HALF],
                        in_=fp[:, 0:HALF], func=Act.Square, scale=LAM)
            for t in range(1, 9):
                nc.vector.tensor_tensor(
                    G[:, 0:FD], G[:, t * FD:(t + 1) * FD], G[:, 0:FD], Alu.add)
            Gs[sign] = G
        # phase 3: low-band passes; keep PSUM tiles for the Ln batch
        for sign in ("A", "B"):
            los[sign] = []
            Wx = xs["WL" + sign]
            for h in range(2):
                sp = psb.tile([128, 512], F32, tag="ps")
                for g in range(3):
                    nc.tensor.matmul(
                        sp[:, 0:HALF],
                        lhsT=KQ96_sb[:, g * 128:(g + 1) * 128],
                        rhs=Wx[:, g * 960 + h * 480:g * 960 + h * 480 + 480
                              ].rearrange("q (a b) -> q a b", b=W_)[:, :15, :WO],
                        start=(g == 0), stop=(g == 2))
                los[sign].append(sp)
        # phase 4: Ln of low band (batched), then solve per sign
        for sign in ("A", "B"):
            LLO = wrk.tile([128, FD], F32, tag="LLO" + sign)
            for h in range(2):
                nc.scalar.activation(
                    out=LLO[:, h * HALF:(h + 1) * HALF],
                    in_=los[sign][h][:, 0:HALF], func=Act.Ln)
            LLOs[sign] = LLO
        args = {}
        for sign in ("A", "B"):
            p1, G = SHs[sign], Gs[sign]
            sq = wrk.tile([128, FD], F32, tag="sq" + sign)
            nc.vector.tensor_tensor(sq[:, :], p1[:, :], p1[:, :], Alu.mult)
            arg = wrk.tile([128, FD], F32, tag="arg" + sign)
            nc.vector.scalar_tensor_tensor(
                out=arg[:, :], in0=G[:, 0:FD], scalar=2.0, in1=sq[:, :],
                op0=Alu.mult, op1=Alu.subtract)
            nc.vector.tensor_scalar(out=arg[:, :], in0=arg[:, :],
                                    scalar1=0.0, scalar2=None, op0=Alu.max)
            args[sign] = arg
        srs = {}
        for sign in ("A", "B"):
            sr = wrk.tile([128, FD], F32, tag="sr" + sign)
            nc.scalar.activation(out=sr[:, :], in_=args[sign][:, :], func=Act.Sqrt)
            srs[sign] = sr
        lhis = {}
        for sign in ("A", "B"):
            aa = wrk.tile([128, FD], F32, tag="aa" + sign)
            nc.vector.tensor_tensor(aa[:, :], SHs[sign][:, :], srs[sign][:, :], Alu.add)
            lhi = wrk.tile([128, FD], F32, tag="lhi" + sign)
            nc.scalar.activation(out=lhi[:, :], in_=aa[:, :], func=Act.Ln)
            lhis[sign] = lhi
        for sign in ("A", "B"):
            lhi, LLO = lhis[sign], LLOs[sign]
            nc.vector.tensor_scalar(out=lhi[:, :], in0=lhi[:, :],
                                    scalar1=1.0 / Q, scalar2=SC_sb[:, 0:1],
                                    op0=Alu.mult, op1=Alu.add)
            nc.vector.tensor_scalar(out=LLO[:, :], in0=LLO[:, :],
                                    scalar1=1.0 / Q, scalar2=SC_sb[:, 1:2],
                                    op0=Alu.mult, op1=Alu.add)
            nc.vector.tensor_tensor(lhi[:, :], LLO[:, :], lhi[:, :], Alu.max)
            nc.vector.tensor_scalar(out=lhi[:, :], in0=lhi[:, :],
                                    scalar1=SC_sb[:, 2:3], scalar2=None,
                                    op0=Alu.max)
        for sign in ("A", "B"):
            mt = mtp.tile([128, FD], F16, tag="m" + sign)
            nc.scalar.activation(out=mt[:, :], in_=lhis[sign][:, :], func=Act.Exp)
            m_out[sign] = mt

        # --- combine: y = (mA - mB) @ [I;-I] + bias, position-major ---
        for c0 in range(0, FD, 128):
            cw = min(128, FD - c0)
            pt = psc.tile([128, COUT], F32)
            nc.tensor.matmul(pt[:cw, :], lhsT=m_out["A"][:, c0:c0 + cw],
                             rhs=M1_sb[:, :], start=True, stop=False)
            nc.tensor.matmul(pt[:cw, :], lhsT=m_out["B"][:, c0:c0 + cw],
                             rhs=M2_sb[:, :], start=False, stop=True)
            ysb = tsb.tile([128, COUT], F32)
            nc.vector.tensor_tensor(ysb[:cw, :], pt[:cw, :], BC_sb[:cw, :], Alu.add)
            nc.sync.dma_start(out=Y[c0:c0 + cw, :], in_=ysb[:cw, :])
    nc.finalize()
    return nc


def _host_prep(x, k1, k2, bias):
    x = np.asarray(x, np.float64)
    K1 = np.exp(np.asarray(k1, np.float64).reshape(9 * C, COUT))
    K2 = np.exp(np.asarray(k2, np.float64).reshape(9 * C, COUT))
    Kmax = np.concatenate([K1.max(axis=0), K2.max(axis=0)])      # [128]
    U = 0.1 * Kmax
    bx_hi = XT_TOP ** (1.0 / Q) / WMAX
    bk = PT_TOP ** (1.0 / Q) / (bx_hi * WMAX * Kmax)             # [128]
    bx_lo = XT_TOP ** (1.0 / Q) / WCLIP
    a_hi = bx_hi * bk
    a_lo = bx_lo * bk

    def bf16_pow(w, bxs):
        z = (bxs * w) ** Q
        return z.astype(ml_dtypes.bfloat16)

    # K-side table [32, 9*128]: KQ[ci, t*128 + col] = (bk*K)^Q
    KK = np.concatenate([K1, K2], axis=1)                        # [288, 128]
    KQv = ((bk[None, :] * KK) ** Q)                              # [288, 128]
    KQt = np.zeros((C, 9 * 128), np.float64)
    for t in range(9):
        KQt[:, t * 128:(t + 1) * 128] = KQv[t * C:(t + 1) * C, :]
    KQt = KQt.astype(ml_dtypes.bfloat16)

    b_hi = (-np.log(2 * LAM) / Q - np.log(a_hi)).astype(np.float32)
    b_lo = (-np.log(a_lo)).astype(np.float32)
    lnU = np.log(U).astype(np.float32)
    SC = np.ascontiguousarray(np.stack([b_hi, b_lo, lnU], axis=1))
    M1 = np.vstack([np.eye(COUT, dtype=np.float16),
                    -np.eye(COUT, dtype=np.float16)])
    M2 = np.ascontiguousarray(-M1)
    BC = np.tile(np.asarray(bias, np.float32).reshape(1, COUT), (128, 1))
    # K=96 lhsT table: KQ96[j*32+ci, g*128+col] = (bk*K)^Q at p=(g*3+j)*32+ci
    KQ96t = np.zeros((96, 3 * 128), np.float64)
    for g in range(3):
        for j in range(3):
            KQ96t[j * C:(j + 1) * C, g * 128:(g + 1) * 128] = \
                KQv[(g * 3 + j) * C:(g * 3 + j + 1) * C, :]
    KQ96t = KQ96t.astype(ml_dtypes.bfloat16)
    shared = dict(KQ=np.ascontiguousarray(KQt), KQ96=np.ascontiguousarray(KQ96t),
                  SC=SC, M1=np.ascontiguousarray(M1),
                  M2=M2, BC=np.ascontiguousarray(BC))

    in_maps = []
    for n in range(N_CORES):
        rows = x[n].reshape(NPIX, C).T                           # [32, 1024]
        wA = np.maximum(rows, 0.0)
        wB = np.maximum(-rows, 0.0)
        m = {}
        for sign, w in (("A", wA), ("B", wB)):
            hi = bf16_pow(w, bx_hi)
            lo = bf16_pow(np.minimum(w, WCLIP), bx_lo)
            e = np.zeros((C, XROW), ml_dtypes.bfloat16)
            o = np.zeros((C, XROW), ml_dtypes.bfloat16)
            e[:, :NPIX] = hi
            o[:, :NPIX - 1] = hi[:, 1:]
            m[f"PH{sign}e"] = e
            m[f"PH{sign}o"] = o
            for band, pw in (("H", hi), ("L", lo)):
                Wt = np.zeros((96, 3 * 960), ml_dtypes.bfloat16)
                for g in range(3):
                    for j in range(3):
                        offp = g * 32 + j
                        Wt[j * C:(j + 1) * C, g * 960:g * 960 + 958] = \
                            pw[:, offp:offp + 958]
                m[f"W{band}{sign}"] = Wt
        in_maps.append({**m, **shared})
    return in_maps


def kernel(x, k1, k2, bias):
    global last_results
    if "nc" not in _cache:
        _cache["nc"] = _build_module()
    nc = _cache["nc"]
    in_maps = _host_prep(x, k1, k2, bias)
    trace = bool(int(os.environ.get("KTRACE", "0")))
    if trace:
        _ensure_axon_ntff_hook()
    res = run_bass_kernel_spmd(
        nc, in_maps, core_ids=list(range(N_CORES)), trace=trace,
    )
    last_results = res
    y = np.stack([r["Y"].reshape(HO, WO, COUT) for r in res.results], axis=0)
    return y.astype(np.float32)


# revision 10
# speedup vs baseline: 1.0656x; 1.0656x over previous
"""Bipolar morphological conv2d for Trainium2 (8 NeuronCores) via p-norm
soft-max on the PE.

Math: y = m(lp1,K1) - m(lp1,K2) - m(lp2,K1) + m(lp2,K2) + bias with
m(logp,k)[c] = exp(max_p(logp_p + k_pc)) = max(U_c, max_p(w_p * K_pc)),
K = exp(k), U_c = .1 max_p K_pc, w = relu(+-x) (entries below U are
subsumed by the U clamp, applied at the end in log domain).

The max over p=288 is approximated by a power-64 p-norm computed as a
MATMUL over host-prepared w^64 patches: S = sum_p (a v_p)^64.  Accuracy
is recovered by a two-term solve using per-tap partial sums F_t:
p1 = sum F_t, p2 = sum F_t^2 (~S128 up to same-tap ties), then the top
term a solves a+b=p1, a^2+b^2=p2: a = (p1 + sqrt(2 p2 - p1^2))/2, and
m = a^(1/64)/alpha.  A second scale band (w clipped at 0.3, plain
p-norm) covers small maxima that underflow the main band; bands merge
in log domain with the exact ln(U) floor.  Simulated end-to-end error
vs the reference: rel L2 ~1.0e-2 (budget 2e-2).

Device (per core = one batch image; partitions = [64c K1 | 64c K2]):
  PE:  per (sign, tap, half): K=32 matmuls vs bf16 w^64 window rows ->
       F_t in PSUM; accumulated passes for S_hi / S_lo.
  ACT: squares F_t into bf16 (scale 1e-19), Ln / Sqrt / Exp of the solve.
  DVE: sums of squares (bf16 2x), the solve arithmetic, band merge.
  PE:  final combine (+-I transpose matmuls) + bias, DMA out.
Host precomputes all powers/scales (input-only transforms), so the
device never exponentiates x.
"""

import os
from contextlib import ExitStack

import numpy as np
import ml_dtypes

import concourse.bass as bass
import concourse.mybir as mybir
from concourse import bacc
import concourse.tile as tile
from concourse.bass_utils import run_bass_kernel_spmd

N_CORES = 8
H = W = C = 32
COUT = 64
HO = WO = 30
NPIX = H * W            # 1024
FD = HO * WO            # 900
XROW = 1056             # padded power-row length
Q = 64
LAM = 1e-19             # F rescale inside ACT Square
XT_TOP = 1e33           # x-side bf16 top target
PT_TOP = 1e36           # per-product fp32 top target
WMAX = 4.8              # |x| bound
WCLIP = 0.3             # low-band clip
HALF = 450              # positions per PSUM half (15 rows)

F32 = mybir.dt.float32
F16 = mybir.dt.float16
BF16 = mybir.dt.bfloat16
_cache: dict = {}
last_results = None


def _ensure_axon_ntff_hook():
    import sys
    import types

    try:
        import antenv.axon_hooks  # noqa: F401
        return
    except ImportError:
        pass
    try:
        mod = types.ModuleType("antenv.axon_hooks")
        holder = [None]
        mod.set_axon_ntff_profile_hook = lambda h: holder.__setitem__(0, h)
        mod.get_axon_ntff_profile_hook = lambda: holder[0]
        sys.modules["antenv.axon_hooks"] = mod
        from trn_agent_boot.trn_boot import _ntff_profile_via_ctypes

        so = "/opt/axon/libaxon_pjrt.so"
        if os.path.exists(so):
            holder[0] = _ntff_profile_via_ctypes(so)
    except Exception:
        pass


def _build_module():
    nc = bacc.Bacc()
    Alu = mybir.AluOpType
    Act = mybir.ActivationFunctionType

    # power rows: per sign/band, even+odd parity copies [C, XROW] bf16
    drams = {}
    for nm in ("PHAe", "PHAo", "PHBe", "PHBo", "PLAe", "PLAo", "PLBe", "PLBo"):
        drams[nm] = nc.dram_tensor(nm, [C, XROW], BF16, kind="ExternalInput")
    KQ = nc.dram_tensor("KQ", [C, 9 * 128], BF16, kind="ExternalInput")
    SC = nc.dram_tensor("SC", [128, 3], F32, kind="ExternalInput")  # b_hi, b_lo, lnU
    M1 = nc.dram_tensor("M1", [128, COUT], F16, kind="ExternalInput")
    M2 = nc.dram_tensor("M2", [128, COUT], F16, kind="ExternalInput")
    BC = nc.dram_tensor("BC", [128, COUT], F32, kind="ExternalInput")
    Y = nc.dram_tensor("Y", [FD, COUT], F32, kind="ExternalOutput")

    with tile.TileContext(nc) as tc, ExitStack() as ctx:
        const = ctx.enter_context(tc.tile_pool(name="const", bufs=1))
        psb = ctx.enter_context(tc.tile_pool(name="psb", bufs=6, space="PSUM"))
        psc = ctx.enter_context(tc.tile_pool(name="psc", bufs=2, space="PSUM"))
        gst = ctx.enter_context(tc.tile_pool(name="gst", bufs=2))
        wrk = ctx.enter_context(tc.tile_pool(name="wrk", bufs=2))
        mtp = ctx.enter_context(tc.tile_pool(name="mtp", bufs=2))
        tsb = ctx.enter_context(tc.tile_pool(name="tsb", bufs=2))

        xs = {}
        for i, nm in enumerate(("PHAe", "PHAo", "PHBe", "PHBo",
                                "PLAe", "PLAo", "PLBe", "PLBo")):
            t = const.tile([C, XROW], BF16, name=nm, tag=nm)
            eng = nc.sync if i % 2 == 0 else nc.gpsimd
            eng.dma_start(out=t[:, :], in_=drams[nm][:, :])
            xs[nm] = t
        KQ_sb = const.tile([C, 9 * 128], BF16)
        nc.gpsimd.dma_start(out=KQ_sb[:, :], in_=KQ[:, :])
        SC_sb = const.tile([128, 3], F32)
        nc.gpsimd.dma_start(out=SC_sb[:, :], in_=SC[:, :])
        M1_sb = const.tile([128, COUT], F16)
        nc.gpsimd.dma_start(out=M1_sb[:, :], in_=M1[:, :])
        M2_sb = const.tile([128, COUT], F16)
        nc.gpsimd.dma_start(out=M2_sb[:, :], in_=M2[:, :])
        BC_sb = const.tile([128, COUT], F32)
        nc.gpsimd.dma_start(out=BC_sb[:, :], in_=BC[:, :])

        def rhs_win(sign, band, t, h):
            """window AP [32, 15, 30] for tap t, half h."""
            i, j = divmod(t, 3)
            off = i * W + j
            key = ("PH" if band == "hi" else "PL") + sign
            if off % 2 == 0:
                src, base = xs[key + "e"], off
            else:
                src, base = xs[key + "o"], off - 1
            base += h * 15 * W
            return src[:, base:base + 480].rearrange(
                "q (a b) -> q a b", b=W)[:, :15, :WO]

        m_out = {}
        SHs, Gs, LLOs, los = {}, {}, {}, {}
        # phase 1: accumulated S passes (hi), staged via DVE with the lam scale
        for sign in ("A", "B"):
            p1 = wrk.tile([128, FD], F32, tag="p1" + sign)
            for h in range(2):
                sp = psb.tile([128, 512], F32, tag="ps")
                for t in range(9):
                    nc.tensor.matmul(
                        sp[:, 0:HALF],
                        lhsT=KQ_sb[:, t * 128:(t + 1) * 128],
                        rhs=rhs_win(sign, "hi", t, h),
                        start=(t == 0), stop=(t == 8))
                nc.vector.tensor_scalar(
                    out=p1[:, h * HALF:(h + 1) * HALF], in0=sp[:, 0:HALF],
                    scalar1=float(LAM), scalar2=None, op0=Alu.mult)
            SHs[sign] = p1
        # phase 2: per-tap F passes, ACT Square only (one table load)
        for sign in ("A", "B"):
            G = gst.tile([128, 9 * FD], BF16, tag="G" + sign)
            for t in range(9):
                for h in range(2):
                    fp = psb.tile([128, 512], F32, tag="ps")
                    nc.tensor.matmul(
                        fp[:, 0:HALF],
                        lhsT=KQ_sb[:, t * 128:(t + 1) * 128],
                        rhs=rhs_win(sign, "hi", t, h),
                        start=True, stop=True)
                    nc.scalar.activation(
                        out=G[:, t * FD + h * HALF:t * FD + (h + 1) * HALF],
                        in_=fp[:, 0:HALF], func=Act.Square, scale=LAM)
            for t in range(1, 9):
                nc.vector.tensor_tensor(
                    G[:, 0:FD], G[:, t * FD:(t + 1) * FD], G[:, 0:FD], Alu.add)
            Gs[sign] = G
        # phase 3: low-band passes; keep PSUM tiles for the Ln batch
        for sign in ("A", "B"):
            los[sign] = []
            for h in range(2):
                sp = psb.tile([128, 512], F32, tag="ps")
                for t in range(9):
                    nc.tensor.matmul(
                        sp[:, 0:HALF],
                        lhsT=KQ_sb[:, t * 128:(t + 1) * 128],
                        rhs=rhs_win(sign, "lo", t, h),
                        start=(t == 0), stop=(t == 8))
                los[sign].append(sp)
        # phase 4: Ln of low band (batched), then solve per sign
        for sign in ("A", "B"):
            LLO = wrk.tile([128, FD], F32, tag="LLO" + sign)
            for h in range(2):
                nc.scalar.activation(
                    out=LLO[:, h * HALF:(h + 1) * HALF],
                    in_=los[sign][h][:, 0:HALF], func=Act.Ln)
            LLOs[sign] = LLO
        args = {}
        for sign in ("A", "B"):
            p1, G = SHs[sign], Gs[sign]
            sq = wrk.tile([128, FD], F32, tag="sq" + sign)
            nc.vector.tensor_tensor(sq[:, :], p1[:, :], p1[:, :], Alu.mult)
            arg = wrk.tile([128, FD], F32, tag="arg" + sign)
            nc.vector.scalar_tensor_tensor(
                out=arg[:, :], in0=G[:, 0:FD], scalar=2.0, in1=sq[:, :],
                op0=Alu.mult, op1=Alu.subtract)
            nc.vector.tensor_scalar(out=arg[:, :], in0=arg[:, :],
                                    scalar1=0.0, scalar2=None, op0=Alu.max)
            args[sign] = arg
        srs = {}
        for sign in ("A", "B"):
            sr = wrk.tile([128, FD], F32, tag="sr" + sign)
            nc.scalar.activation(out=sr[:, :], in_=args[sign][:, :], func=Act.Sqrt)
            srs[sign] = sr
        lhis = {}
        for sign in ("A", "B"):
            aa = wrk.tile([128, FD], F32, tag="aa" + sign)
            nc.vector.tensor_tensor(aa[:, :], SHs[sign][:, :], srs[sign][:, :], Alu.add)
            lhi = wrk.tile([128, FD], F32, tag="lhi" + sign)
            nc.scalar.activation(out=lhi[:, :], in_=aa[:, :], func=Act.Ln)
            lhis[sign] = lhi
        for sign in ("A", "B"):
            lhi, LLO = lhis[sign], LLOs[sign]
            nc.vector.tensor_scalar(out=lhi[:, :], in0=lhi[:, :],
                                    scalar1=1.0 / Q, scalar2=SC_sb[:, 0:1],
                                    op0=Alu.mult, op1=Alu.add)
            nc.vector.tensor_scalar(out=LLO[:, :], in0=LLO[:, :],
                                    scalar1=1.0 / Q, scalar2=SC_sb[:, 1:2],
                                    op0=Alu.mult, op1=Alu.add)
            nc.vector.tensor_tensor(lhi[:, :], LLO[:, :], lhi[:, :], Alu.max)
            nc.vector.tensor_scalar(out=lhi[:, :], in0=lhi[:, :],
                                    scalar1=SC_sb[:, 2:3], scalar2=None,
                                    op0=Alu.max)
        for sign in ("A", "B"):
            mt = mtp.tile([128, FD], F16, tag="m" + sign)
            nc.scalar.activation(out=mt[:, :], in_=lhis[sign][:, :], func=Act.Exp)
            m_out[sign] = mt

        # --- combine: y = (mA - mB) @ [I;-I] + bias, position-major ---
        for c0 in range(0, FD, 128):
            cw = min(128, FD - c0)
            pt = psc.tile([128, COUT], F32)
            nc.tensor.matmul(pt[:cw, :], lhsT=m_out["A"][:, c0:c0 + cw],
                             rhs=M1_sb[:, :], start=True, stop=False)
            nc.tensor.matmul(pt[:cw, :], lhsT=m_out["B"][:, c0:c0 + cw],
                             rhs=M2_sb[:, :], start=False, stop=True)
            ysb = tsb.tile([128, COUT], F32)
            nc.vector.tensor_tensor(ysb[:cw, :], pt[:cw, :], BC_sb[:cw, :], Alu.add)
            nc.sync.dma_start(out=Y[c0:c0 + cw, :], in_=ysb[:cw, :])
    nc.finalize()
    return nc


def _host_prep(x, k1, k2, bias):
    x = np.asarray(x, np.float64)
    K1 = np.exp(np.asarray(k1, np.float64).reshape(9 * C, COUT))
    K2 = np.exp(np.asarray(k2, np.float64).reshape(9 * C, COUT))
    Kmax = np.concatenate([K1.max(axis=0), K2.max(axis=0)])      # [128]
    U = 0.1 * Kmax
    bx_hi = XT_TOP ** (1.0 / Q) / WMAX
    bk = PT_TOP ** (1.0 / Q) / (bx_hi * WMAX * Kmax)             # [128]
    bx_lo = XT_TOP ** (1.0 / Q) / WCLIP
    a_hi = bx_hi * bk
    a_lo = bx_lo * bk

    def bf16_pow(w, bxs):
        z = (bxs * w) ** Q
        return z.astype(ml_dtypes.bfloat16)

    # K-side table [32, 9*128]: KQ[ci, t*128 + col] = (bk*K)^Q
    KK = np.concatenate([K1, K2], axis=1)                        # [288, 128]
    KQv = ((bk[None, :] * KK) ** Q)                              # [288, 128]
    KQt = np.zeros((C, 9 * 128), np.float64)
    for t in range(9):
        KQt[:, t * 128:(t + 1) * 128] = KQv[t * C:(t + 1) * C, :]
    KQt = KQt.astype(ml_dtypes.bfloat16)

    b_hi = (-np.log(2 * LAM) / Q - np.log(a_hi)).astype(np.float32)
    b_lo = (-np.log(a_lo)).astype(np.float32)
    lnU = np.log(U).astype(np.float32)
    SC = np.ascontiguousarray(np.stack([b_hi, b_lo, lnU], axis=1))
    M1 = np.vstack([np.eye(COUT, dtype=np.float16),
                    -np.eye(COUT, dtype=np.float16)])
    M2 = np.ascontiguousarray(-M1)
    BC = np.tile(np.asarray(bias, np.float32).reshape(1, COUT), (128, 1))
    shared = dict(KQ=np.ascontiguousarray(KQt), SC=SC, M1=np.ascontiguousarray(M1),
                  M2=M2, BC=np.ascontiguousarray(BC))

    in_maps = []
    for n in range(N_CORES):
        rows = x[n].reshape(NPIX, C).T                           # [32, 1024]
        wA = np.maximum(rows, 0.0)
        wB = np.maximum(-rows, 0.0)
        m = {}
        for sign, w in (("A", wA), ("B", wB)):
            hi = bf16_pow(w, bx_hi)
            lo = bf16_pow(np.minimum(w, WCLIP), bx_lo)
            for band, pw in (("H", hi), ("L", lo)):
                e = np.zeros((C, XROW), ml_dtypes.bfloat16)
                o = np.zeros((C, XROW), ml_dtypes.bfloat16)
                e[:, :NPIX] = pw
                o[:, :NPIX - 1] = pw[:, 1:]
                m[f"P{band}{sign}e"] = e
                m[f"P{band}{sign}o"] = o
        in_maps.append({**m, **shared})
    return in_maps


def kernel(x, k1, k2, bias):
    global last_results
    if "nc" not in _cache:
        _cache["nc"] = _build_module()
    nc = _cache["nc"]
    in_maps = _host_prep(x, k1, k2, bias)
    trace = bool(int(os.environ.get("KTRACE", "0")))
    if trace:
        _ensure_axon_ntff_hook()
    res = run_bass_kernel_spmd(
        nc, in_maps, core_ids=list(range(N_CORES)), trace=trace,
    )
    last_results = res
    y = np.stack([r["Y"].reshape(HO, WO, COUT) for r in res.results], axis=0)
    return y.astype(np.float32)


# revision 11
# speedup vs baseline: 1.1530x; 1.0820x over previous
"""Bipolar morphological conv2d for Trainium2 (8 NeuronCores) via p-norm
soft-max on the PE.

Math: y = m(lp1,K1) - m(lp1,K2) - m(lp2,K1) + m(lp2,K2) + bias with
m(logp,k)[c] = exp(max_p(logp_p + k_pc)) = max(U_c, max_p(w_p * K_pc)),
K = exp(k), U_c = .1 max_p K_pc, w = relu(+-x) (entries below U are
subsumed by the U clamp, applied at the end in log domain).

The max over p=288 is approximated by a power-64 p-norm computed as a
MATMUL over host-prepared w^64 patches: S = sum_p (a v_p)^64.  Accuracy
is recovered by a two-term solve using per-tap partial sums F_t:
p1 = sum F_t, p2 = sum F_t^2 (~S128 up to same-tap ties), then the top
term a solves a+b=p1, a^2+b^2=p2: a = (p1 + sqrt(2 p2 - p1^2))/2, and
m = a^(1/64)/alpha.  A second scale band (w clipped at 0.3, plain
p-norm) covers small maxima that underflow the main band; bands merge
in log domain with the exact ln(U) floor.  Simulated end-to-end error
vs the reference: rel L2 ~1.0e-2 (budget 2e-2).

Device (per core = one batch image; partitions = [64c K1 | 64c K2]):
  PE:  per (sign, tap, half): K=32 matmuls vs bf16 w^64 window rows ->
       F_t in PSUM; accumulated passes for S_hi / S_lo.
  ACT: squares F_t into bf16 (scale 1e-19), Ln / Sqrt / Exp of the solve.
  DVE: sums of squares (bf16 2x), the solve arithmetic, band merge.
  PE:  final combine (+-I transpose matmuls) + bias, DMA out.
Host precomputes all powers/scales (input-only transforms), so the
device never exponentiates x.
"""

import os
from contextlib import ExitStack

import numpy as np
import ml_dtypes

import concourse.bass as bass
import concourse.mybir as mybir
from concourse import bacc
import concourse.tile as tile
from concourse.bass_utils import run_bass_kernel_spmd

N_CORES = 8
H = W = C = 32
COUT = 64
HO = WO = 30
NPIX = H * W            # 1024
FD = HO * WO            # 900
XROW = 1056             # padded power-row length
Q = 64
LAM = 1e-19             # F rescale inside ACT Square
XT_TOP = 1e33           # x-side bf16 top target
PT_TOP = 1e36           # per-product fp32 top target
WMAX = 4.8              # |x| bound
WCLIP = 0.3             # low-band clip
HALF = 450              # positions per PSUM half (15 rows)

F32 = mybir.dt.float32
F16 = mybir.dt.float16
BF16 = mybir.dt.bfloat16
_cache: dict = {}
last_results = None


def _ensure_axon_ntff_hook():
    import sys
    import types

    try:
        import antenv.axon_hooks  # noqa: F401
        return
    except ImportError:
        pass
    try:
        mod = types.ModuleType("antenv.axon_hooks")
        holder = [None]
        mod.set_axon_ntff_profile_hook = lambda h: holder.__setitem__(0, h)
        mod.get_axon_ntff_profile_hook = lambda: holder[0]
        sys.modules["antenv.axon_hooks"] = mod
        from trn_agent_boot.trn_boot import _ntff_profile_via_ctypes

        so = "/opt/axon/libaxon_pjrt.so"
        if os.path.exists(so):
            holder[0] = _ntff_profile_via_ctypes(so)
    except Exception:
        pass


def _build_module():
    nc = bacc.Bacc()
    Alu = mybir.AluOpType
    Act = mybir.ActivationFunctionType

    # power rows: per sign/band, even+odd parity copies [C, XROW] bf16
    drams = {}
    for nm in ("PHAe", "PHAo", "PHBe", "PHBo", "PLAe", "PLAo", "PLBe", "PLBo"):
        drams[nm] = nc.dram_tensor(nm, [C, XROW], BF16, kind="ExternalInput")
    KQ = nc.dram_tensor("KQ", [C, 9 * 128], BF16, kind="ExternalInput")
    SC = nc.dram_tensor("SC", [128, 3], F32, kind="ExternalInput")  # b_hi, b_lo, lnU
    M1 = nc.dram_tensor("M1", [128, COUT], F16, kind="ExternalInput")
    M2 = nc.dram_tensor("M2", [128, COUT], F16, kind="ExternalInput")
    BC = nc.dram_tensor("BC", [128, COUT], F32, kind="ExternalInput")
    Y = nc.dram_tensor("Y", [FD, COUT], F32, kind="ExternalOutput")

    with tile.TileContext(nc) as tc, ExitStack() as ctx:
        const = ctx.enter_context(tc.tile_pool(name="const", bufs=1))
        psb = ctx.enter_context(tc.tile_pool(name="psb", bufs=3, space="PSUM"))
        psc = ctx.enter_context(tc.tile_pool(name="psc", bufs=2, space="PSUM"))
        gst = ctx.enter_context(tc.tile_pool(name="gst", bufs=2))
        wrk = ctx.enter_context(tc.tile_pool(name="wrk", bufs=2))
        mtp = ctx.enter_context(tc.tile_pool(name="mtp", bufs=2))
        tsb = ctx.enter_context(tc.tile_pool(name="tsb", bufs=2))

        xs = {}
        for i, nm in enumerate(("PHAe", "PHAo", "PHBe", "PHBo",
                                "PLAe", "PLAo", "PLBe", "PLBo")):
            t = const.tile([C, XROW], BF16, name=nm, tag=nm)
            eng = nc.sync if i % 2 == 0 else nc.gpsimd
            eng.dma_start(out=t[:, :], in_=drams[nm][:, :])
            xs[nm] = t
        KQ_sb = const.tile([C, 9 * 128], BF16)
        nc.gpsimd.dma_start(out=KQ_sb[:, :], in_=KQ[:, :])
        SC_sb = const.tile([128, 3], F32)
        nc.gpsimd.dma_start(out=SC_sb[:, :], in_=SC[:, :])
        M1_sb = const.tile([128, COUT], F16)
        nc.gpsimd.dma_start(out=M1_sb[:, :], in_=M1[:, :])
        M2_sb = const.tile([128, COUT], F16)
        nc.gpsimd.dma_start(out=M2_sb[:, :], in_=M2[:, :])
        BC_sb = const.tile([128, COUT], F32)
        nc.gpsimd.dma_start(out=BC_sb[:, :], in_=BC[:, :])

        def rhs_win(sign, band, t, h):
            """window AP [32, 15, 30] for tap t, half h."""
            i, j = divmod(t, 3)
            off = i * W + j
            key = ("PH" if band == "hi" else "PL") + sign
            if off % 2 == 0:
                src, base = xs[key + "e"], off
            else:
                src, base = xs[key + "o"], off - 1
            base += h * 15 * W
            return src[:, base:base + 480].rearrange(
                "q (a b) -> q a b", b=W)[:, :15, :WO]

        m_out = {}
        SHs, Gs, LLOs, los = {}, {}, {}, {}
        HOFF = 512   # bank-aligned second-half column offset in PSUM tiles

        def halves_view(t):   # [128, 2, 450] strided view of a [128,1024] tile
            return t[:, 0:2 * HOFF].rearrange("q (a b) -> q a b", b=HOFF)[:, :, :HALF]

        # phase 1: accumulated S passes (hi), staged via DVE with the lam scale
        for sign in ("A", "B"):
            p1 = wrk.tile([128, FD], F32, tag="p1" + sign)
            sp = psb.tile([128, 1024], F32, tag="ps")
            for h in range(2):
                for t in range(9):
                    nc.tensor.matmul(
                        sp[:, h * HOFF:h * HOFF + HALF],
                        lhsT=KQ_sb[:, t * 128:(t + 1) * 128],
                        rhs=rhs_win(sign, "hi", t, h),
                        start=(t == 0), stop=(t == 8))
            nc.vector.tensor_scalar(
                out=p1[:, :].rearrange("q (a b) -> q a b", b=HALF),
                in0=halves_view(sp),
                scalar1=float(LAM), scalar2=None, op0=Alu.mult)
            SHs[sign] = p1
        # phase 2: per-tap F passes, one ACT Square per tap
        for sign in ("A", "B"):
            G = gst.tile([128, 9 * FD], BF16, tag="G" + sign)
            for t in range(9):
                fp = psb.tile([128, 1024], F32, tag="ps")
                for h in range(2):
                    nc.tensor.matmul(
                        fp[:, h * HOFF:h * HOFF + HALF],
                        lhsT=KQ_sb[:, t * 128:(t + 1) * 128],
                        rhs=rhs_win(sign, "hi", t, h),
                        start=True, stop=True)
                nc.scalar.activation(
                    out=G[:, t * FD:(t + 1) * FD].rearrange(
                        "q (a b) -> q a b", b=HALF),
                    in_=halves_view(fp), func=Act.Square, scale=LAM)
            for t in range(1, 9):
                nc.vector.tensor_tensor(
                    G[:, 0:FD], G[:, t * FD:(t + 1) * FD], G[:, 0:FD], Alu.add)
            Gs[sign] = G
        # phase 3: low-band accumulated passes; PSUM kept for the Ln batch
        for sign in ("A", "B"):
            sp = psb.tile([128, 1024], F32, tag="ps")
            for h in range(2):
                for t in range(9):
                    nc.tensor.matmul(
                        sp[:, h * HOFF:h * HOFF + HALF],
                        lhsT=KQ_sb[:, t * 128:(t + 1) * 128],
                        rhs=rhs_win(sign, "lo", t, h),
                        start=(t == 0), stop=(t == 8))
            los[sign] = sp
        # phase 4: solve DVE part: sq, arg, relu clamp
        args = {}
        for sign in ("A", "B"):
            p1, G = SHs[sign], Gs[sign]
            sq = wrk.tile([128, FD], F32, tag="sq" + sign)
            nc.vector.tensor_tensor(sq[:, :], p1[:, :], p1[:, :], Alu.mult)
            arg = wrk.tile([128, FD], F32, tag="arg" + sign)
            nc.vector.scalar_tensor_tensor(
                out=arg[:, :], in0=G[:, 0:FD], scalar=2.0, in1=sq[:, :],
                op0=Alu.mult, op1=Alu.subtract)
            nc.vector.tensor_scalar(out=arg[:, :], in0=arg[:, :],
                                    scalar1=0.0, scalar2=None, op0=Alu.max)
            args[sign] = arg
        # phase 5: ACT Sqrt batch
        srs = {}
        for sign in ("A", "B"):
            sr = wrk.tile([128, FD], F32, tag="sr" + sign)
            nc.scalar.activation(out=sr[:, :], in_=args[sign][:, :], func=Act.Sqrt)
            srs[sign] = sr
        # phase 6: aa = p1 + sr
        aas = {}
        for sign in ("A", "B"):
            aa = wrk.tile([128, FD], F32, tag="aa" + sign)
            nc.vector.tensor_tensor(aa[:, :], SHs[sign][:, :], srs[sign][:, :],
                                    Alu.add)
            aas[sign] = aa
        # phase 7: ACT Ln batch (lo from PSUM, hi from aa)
        lhis = {}
        for sign in ("A", "B"):
            LLO = wrk.tile([128, FD], F32, tag="LLO" + sign)
            nc.scalar.activation(
                out=LLO[:, :].rearrange("q (a b) -> q a b", b=HALF),
                in_=halves_view(los[sign]), func=Act.Ln)
            LLOs[sign] = LLO
        for sign in ("A", "B"):
            lhi = wrk.tile([128, FD], F32, tag="lhi" + sign)
            nc.scalar.activation(out=lhi[:, :], in_=aas[sign][:, :], func=Act.Ln)
            lhis[sign] = lhi
        # phase 8: log-domain merge: max(lhi+SC0, LLO, SC1); Exp(x/Q + SC2)
        for sign in ("A", "B"):
            lhi, LLO = lhis[sign], LLOs[sign]
            nc.vector.scalar_tensor_tensor(
                out=lhi[:, :], in0=lhi[:, :], scalar=SC_sb[:, 0:1], in1=LLO[:, :],
                op0=Alu.add, op1=Alu.max)
            nc.vector.tensor_scalar(out=lhi[:, :], in0=lhi[:, :],
                                    scalar1=SC_sb[:, 1:2], scalar2=None,
                                    op0=Alu.max)
        for sign in ("A", "B"):
            mt = mtp.tile([128, FD], F16, tag="m" + sign)
            nc.scalar.activation(out=mt[:, :], in_=lhis[sign][:, :], func=Act.Exp,
                                 scale=1.0 / Q, bias=SC_sb[:, 2:3])
            m_out[sign] = mt

        # --- combine: y = (mA - mB) @ [I;-I] + bias, position-major ---
        for c0 in range(0, FD, 128):
            cw = min(128, FD - c0)
            pt = psc.tile([128, COUT], F32)
            nc.tensor.matmul(pt[:cw, :], lhsT=m_out["A"][:, c0:c0 + cw],
                             rhs=M1_sb[:, :], start=True, stop=False)
            nc.tensor.matmul(pt[:cw, :], lhsT=m_out["B"][:, c0:c0 + cw],
                             rhs=M2_sb[:, :], start=False, stop=True)
            ysb = tsb.tile([128, COUT], F32)
            nc.vector.tensor_tensor(ysb[:cw, :], pt[:cw, :], BC_sb[:cw, :], Alu.add)
            nc.sync.dma_start(out=Y[c0:c0 + cw, :], in_=ysb[:cw, :])
    nc.finalize()
    return nc


def _host_prep(x, k1, k2, bias):
    x = np.asarray(x, np.float64)
    K1 = np.exp(np.asarray(k1, np.float64).reshape(9 * C, COUT))
    K2 = np.exp(np.asarray(k2, np.float64).reshape(9 * C, COUT))
    Kmax = np.concatenate([K1.max(axis=0), K2.max(axis=0)])      # [128]
    U = 0.1 * Kmax
    bx_hi = XT_TOP ** (1.0 / Q) / WMAX
    bk = PT_TOP ** (1.0 / Q) / (bx_hi * WMAX * Kmax)             # [128]
    bx_lo = XT_TOP ** (1.0 / Q) / WCLIP
    a_hi = bx_hi * bk
    a_lo = bx_lo * bk

    def bf16_pow(w, bxs):
        z = (bxs * w) ** Q
        return z.astype(ml_dtypes.bfloat16)

    # K-side table [32, 9*128]: KQ[ci, t*128 + col] = (bk*K)^Q
    KK = np.concatenate([K1, K2], axis=1)                        # [288, 128]
    KQv = ((bk[None, :] * KK) ** Q)                              # [288, 128]
    KQt = np.zeros((C, 9 * 128), np.float64)
    for t in range(9):
        KQt[:, t * 128:(t + 1) * 128] = KQv[t * C:(t + 1) * C, :]
    KQt = KQt.astype(ml_dtypes.bfloat16)

    b_hi = -np.log(2 * LAM) / Q - np.log(a_hi)
    b_lo = -np.log(a_lo)
    lnU = np.log(U)
    SC = np.ascontiguousarray(np.stack(
        [Q * (b_hi - b_lo), Q * lnU - Q * b_lo, b_lo], axis=1).astype(np.float32))
    M1 = np.vstack([np.eye(COUT, dtype=np.float16),
                    -np.eye(COUT, dtype=np.float16)])
    M2 = np.ascontiguousarray(-M1)
    BC = np.tile(np.asarray(bias, np.float32).reshape(1, COUT), (128, 1))
    shared = dict(KQ=np.ascontiguousarray(KQt), SC=SC, M1=np.ascontiguousarray(M1),
                  M2=M2, BC=np.ascontiguousarray(BC))

    in_maps = []
    for n in range(N_CORES):
        rows = x[n].reshape(NPIX, C).T                           # [32, 1024]
        wA = np.maximum(rows, 0.0)
        wB = np.maximum(-rows, 0.0)
        m = {}
        for sign, w in (("A", wA), ("B", wB)):
            hi = bf16_pow(w, bx_hi)
            lo = bf16_pow(np.minimum(w, WCLIP), bx_lo)
            for band, pw in (("H", hi), ("L", lo)):
                e = np.zeros((C, XROW), ml_dtypes.bfloat16)
                o = np.zeros((C, XROW), ml_dtypes.bfloat16)
                e[:, :NPIX] = pw
                o[:, :NPIX - 1] = pw[:, 1:]
                m[f"P{band}{sign}e"] = e
                m[f"P{band}{sign}o"] = o
        in_maps.append({**m, **shared})
    return in_maps


def kernel(x, k1, k2, bias):
    global last_results
    if "nc" not in _cache:
        _cache["nc"] = _build_module()
    nc = _cache["nc"]
    in_maps = _host_prep(x, k1, k2, bias)
    trace = bool(int(os.environ.get("KTRACE", "0")))
    if trace:
        _ensure_axon_ntff_hook()
    res = run_bass_kernel_spmd(
        nc, in_maps, core_ids=list(range(N_CORES)), trace=trace,
    )
    last_results = res
    y = np.stack([r["Y"].reshape(HO, WO, COUT) for r in res.results], axis=0)
    return y.astype(np.float32)
